# revision 25
# baseline (speedup 1.0000x reference)
"""GQA attention kernel for Trainium2, 8-core sequence-parallel SPMD.

Model: d_model=1024, 16 q-heads / 4 kv-heads of dim 64, seq 4096, batch 1.

Per-core split: core c handles query rows [512c, 512c+512) for ALL 16 heads,
and (redundantly) computes the full K/V projections. No collectives needed;
the host concatenates the 8 per-core [512, 1024] outputs.

v2: all input staging (fp32->fp16 cast, x transpose, Wq/Wo head-pair
shuffles) happens on the HOST in numpy; the device receives fp16 tensors in
their final SBUF layouts and just DMA-loads them.  Emission order interleaves
the second half of the projections with the first attention slots so ScalarE
(softmax exp, the critical engine) starts as early as possible.

Layout strategy ("transposed scores"):
  - xT [dm, seq] fp16 loaded directly (host pre-transposed).
  - kT[d, seq] = Wk^T @ x^T, qT[d, q] = Wq^T @ xq^T, v[seq, d] = x @ Wv
    (ones-augmented with a 65th column for softmax denominators).
  - scoresT[k, q] = kT^T(slice) @ qT: two K=64 matmuls row-packed into the
    128x128 PE array (q-head pairs chosen cross-kv so each head's kv slice
    naturally sits in the right partition half) -> concurrent on sub-arrays.
  - exp on ScalarE straight out of PSUM (scores bounded ~|3.4|, no max pass),
    fp16 attn written to SBUF.
  - contextT[d(+sum), q] accumulated over 32 k-chunks; row 64 = softmax
    denominator. Normalize with approx-reciprocal + gpsimd broadcast + DVE.
  - out = contextT^T @ Wo + bo accumulated over 8 shuffled d-chunks.
"""

import sys
import numpy as np

sys.path.insert(0, "/opt/trn_rl_repo")

from contextlib import ExitStack  # noqa: E402

import concourse.bass as bass  # noqa: E402
import concourse.bacc as bacc  # noqa: E402
import concourse.tile as tile  # noqa: E402
from concourse import mybir  # noqa: E402
from concourse.bass_utils import run_bass_kernel_spmd  # noqa: E402

N_CORES = 8
SEQ = 4096
DM = 1024
QS = SEQ // N_CORES  # 512 query rows per core
HD = 64
NQ = 16
NKV = 4
KV = NKV * HD  # 256
CC = DM // 128  # 8 contraction chunks
KC = SEQ // 128  # 32 key chunks
QT = QS // 128  # 4 query row tiles
F16 = mybir.dt.float16
F32 = mybir.dt.float32
I32 = mybir.dt.int32
F8 = mybir.dt.float8e4
ts = bass.ts

# DVE fast-exp2: attn = bitcast_f32(int32(score*EXP_SCALE + EXP_OFFSET)).
# EXP_SCALE folds the 1/sqrt(d) softmax scale and log2(e) into the fp32
# exponent/mantissa construction; EXP_OFFSET carries the exponent bias with
# the balanced magic constant (max rel err ~3% on the affected tiles).
EXP_SCALE = float(0.125 * np.log2(np.e) * (1 << 23))
EXP_OFFSET = float((127.0 - 0.0434) * (1 << 23))

_CACHE = {}


def _emit(tc: tile.TileContext):
    nc = tc.nc
    # All inputs pre-laid-out on host, fp16.
    xqt = nc.dram_tensor("xqt", [128, CC, QS], F16, kind="ExternalInput").ap()
    Wq = nc.dram_tensor("wq", [128, CC, DM], F16, kind="ExternalInput").ap()
    bq2 = nc.dram_tensor("bq2", [128, CC], F32, kind="ExternalInput").ap()
    Wk = nc.dram_tensor("wk", [128, CC, KV], F16, kind="ExternalInput").ap()
    bk2 = nc.dram_tensor("bk2", [128, 2], F32, kind="ExternalInput").ap()
    Wv = nc.dram_tensor("wv", [128, CC, KV], F16, kind="ExternalInput").ap()
    bv = nc.dram_tensor("bv", [1, KV], F16, kind="ExternalInput").ap()
    Wo = nc.dram_tensor("wo", [128, CC, DM], F16, kind="ExternalInput").ap()
    bo = nc.dram_tensor("bo", [1, DM], F16, kind="ExternalInput").ap()
    out = nc.dram_tensor("out", [QS, DM], F32, kind="ExternalOutput").ap()

    stack = ExitStack()
    with stack:
        consts = stack.enter_context(tc.tile_pool(name="consts", bufs=1))
        # ---- weight/bias loads (already fp16, final layout) ----
        wk_sb = consts.tile([128, CC, KV], F16)
        bk_sb = consts.tile([128, 2], F32)
        wv_sb = consts.tile([128, CC, KV], F16)
        bv_sb = consts.tile([1, KV], F16)
        wq_sb = consts.tile([128, CC, DM], F16)
        bq_sb = consts.tile([128, CC], F32)
        wo_sb = consts.tile([128, CC, DM], F16)
        bo_sb = consts.tile([1, DM], F16)
        ones_sb = consts.tile([1, 512], F16)
        nc.vector.memset(ones_sb[:], 1.0)

# persistent activations
        acts = stack.enter_context(tc.tile_pool(name="acts", bufs=1))
        xqt_sb = acts.tile([128, CC, QS], F16)
        kt_sb = acts.tile([128, 2, SEQ], F16)      # kv dims (pairs) x seq
        # [kc-pair, pair-parity, kv head, d(+1, pad to 68)] fp8 for DoubleRow
        v_sb = acts.tile([128, KC // 2, 2, NKV, 68], F8)
        qt_sb = acts.tile([128, CC, QS], F16)      # shuffled q dims x q-rows
        ctxt_sb = acts.tile([128, CC, QS], F16)
        kt_loc = acts.tile([128, 2, QS], F16)
        v_loc = acts.tile([128, 2, 4, 2, 68], F8)  # [gpair, m, g, d+1pad]
        nc.gpsimd.memset(v_loc[:, :, :, :, HD], 1.0)

        # DMA priority: sync queue carries the k/v local projection critical
        # path, gpsimd queue the q path, scalar queue the (late-needed)
        # out-proj weights.
        nc.sync.dma_start(wk_sb[:], Wk)
        nc.sync.dma_start(bk_sb[:], bk2)
        nc.sync.dma_start(xqt_sb[:], xqt)
        nc.sync.dma_start(wv_sb[:], Wv)
        nc.sync.dma_start(bv_sb[:], bv)
        nc.sync.dma_start(bq_sb[:], bq2)
        for cc in range(CC):
            nc.sync.dma_start(wq_sb[:, cc, :], Wq[:, cc, :])
        for cc in range(CC):
            nc.scalar.dma_start(wo_sb[:, cc, :], Wo[:, cc, :])
        nc.scalar.dma_start(bo_sb[:], bo)

        # ---- phase 1: local projections + k/v AllGather ----
        # Each core projects only its own 512 rows of x; kT/v slices for the
        # other 7/8 of the sequence come from an on-chip AllGather instead of
        # being recomputed 8x.
        dramp = stack.enter_context(tc.tile_pool(name="dram", bufs=1, space="DRAM"))
        CCW = QS + 4 * 2 * 68 // 2  # kt pair slice + 2 v heads (fp8 packed)
        cc_in1 = dramp.tile([128, CCW], F16)
        cc_in2 = dramp.tile([128, CCW], F16)
        cc_out1 = dramp.tile([N_CORES, 128, CCW], F16)
        cc_out2 = dramp.tile([N_CORES, 128, CCW], F16)

        with tc.tile_pool(name="proj_ps", bufs=2, space="PSUM") as projp:
            # local kT slice: [128 dims of kv-head pair (2j, 2j+1), own 512]
            def kproj_loc(j):
                ps = projp.tile([128, 512], F32, tag="proj")
                for cc in range(CC):
                    nc.tensor.matmul(
                        ps[:], wk_sb[:, cc, ts(j, 128)], xqt_sb[:, cc, :],
                        start=(cc == 0), stop=(cc == CC - 1),
                    )
                nc.vector.tensor_scalar(
                    out=kt_loc[:, j, :], in0=ps[:],
                    scalar1=bk_sb[:, j : j + 1], scalar2=None,
                    op0=mybir.AluOpType.add,
                )

            # local v slice: 4 chunks of [128 rows, 4 kv heads x 64] + ones
            def vproj_loc(m):
                ps = projp.tile([128, 512], F32, tag="proj")
                nc.tensor.matmul(
                    ps[:, 0:KV], ones_sb[0:1, 0:128], bv_sb[0:1, :],
                    start=True, stop=False,
                )
                for cc in range(CC):
                    nc.tensor.matmul(
                        ps[:, 0:KV], xqt_sb[:, cc, ts(m, 128)], wv_sb[:, cc, :],
                        start=False, stop=(cc == CC - 1),
                    )
                nc.vector.tensor_copy(
                    out=v_loc[:, :, m, :, 0:HD],
                    in_=ps[:, 0:KV].rearrange("p (gp g d) -> p gp g d", gp=2, g=2),
                )

            kproj_loc(0)
            for m in range(4):
                vproj_loc(m)
            nc.sync.dma_start(cc_in1[:, 0:QS], kt_loc[:, 0, :])
            nc.sync.dma_start(
                cc_in1[:, QS:],
                v_loc[:, 0].rearrange("p m g d -> p (m g d)").bitcast(F16),
            )
            nc.gpsimd.collective_compute(
                "AllGather",
                mybir.AluOpType.bypass,
                replica_groups=[list(range(N_CORES))],
                ins=[cc_in1.opt()],
                outs=[cc_out1.opt()],
            )
            for c in range(N_CORES):
                nc.scalar.dma_start(kt_sb[:, 0, ts(c, QS)], cc_out1[c, :, 0:QS])
                nc.scalar.dma_start(
                    v_sb[:, 2 * c : 2 * c + 2, :, 0:2, :].bitcast(F16),
                    cc_out1[c, :, QS:].rearrange(
                        "p (k j g e) -> p k j g e", k=2, j=2, g=2
                    ),
                )

            # q projection overlaps the first collective
            def qproj(s):
                ps = projp.tile([128, 512], F32, tag="proj")
                for cc in range(CC):
                    nc.tensor.matmul(
                        ps[:], wq_sb[:, cc, ts(s, 128)], xqt_sb[:, cc, :],
                        start=(cc == 0), stop=(cc == CC - 1),
                    )
                nc.vector.tensor_scalar(
                    out=qt_sb[:, s, :], in0=ps[:],
                    scalar1=bq_sb[:, s : s + 1], scalar2=None,
                    op0=mybir.AluOpType.add,
                )

            for s in range(8):
                qproj(s)

            kproj_loc(1)
            nc.sync.dma_start(cc_in2[:, 0:QS], kt_loc[:, 1, :])
            nc.sync.dma_start(
                cc_in2[:, QS:],
                v_loc[:, 1].rearrange("p m g d -> p (m g d)").bitcast(F16),
            )
            nc.gpsimd.collective_compute(
                "AllGather",
                mybir.AluOpType.bypass,
                replica_groups=[list(range(N_CORES))],
                ins=[cc_in2.opt()],
                outs=[cc_out2.opt()],
            )
            for c in range(N_CORES):
                nc.scalar.dma_start(kt_sb[:, 1, ts(c, QS)], cc_out2[c, :, 0:QS])
                nc.scalar.dma_start(
                    v_sb[:, 2 * c : 2 * c + 2, :, 2:4, :].bitcast(F16),
                    cc_out2[c, :, QS:].rearrange(
                        "p (k j g e) -> p k j g e", k=2, j=2, g=2
                    ),
                )

        # ---- phase 2: attention (scores triple-buffered, AV lags 2 kc) ----
        with (
            tc.tile_pool(name="ctx_ps", bufs=2, space="PSUM") as ctxp,
            tc.tile_pool(name="scores_ps", bufs=1, space="PSUM") as scoresp,
            tc.tile_pool(name="attn", bufs=5) as attnp,
            tc.tile_pool(name="it", bufs=2) as itp,
            tc.tile_pool(name="norm", bufs=2) as normp,
            tc.tile_pool(name="cs", bufs=4) as csp,
            tc.tile_pool(name="odd", bufs=2) as oddp,
            tc.tile_pool(name="out_sb", bufs=2) as outsb,
        ):
            sc_pair = scoresp.tile([128, 2, 1024], F32)
            sc_dve = scoresp.tile([128, 1024], F32)
            at_big = attnp.tile([128, 3, 2, 1024], F8)

            def attn_slot(s):
                g2, _i = divmod(s, 4)
                ctx_a = ctxp.tile([HD + 1, QS], F32, tag="ctx")
                ctx_b = ctxp.tile([HD + 1, QS], F32, tag="ctx")

                def av(kcp):
                    at2 = at_big[:, kcp % 3, :, :]
                    nc.tensor.matmul(
                        ctx_a[:], v_sb[:, kcp, :, 2 * g2, 0 : HD + 1],
                        at2[:, :, 0:512],
                        perf_mode=mybir.MatmulPerfMode.DoubleRow,
                        start=(kcp == 0), stop=(kcp == KC // 2 - 1),
                        skip_group_check=True,
                    )
                    nc.tensor.matmul(
                        ctx_b[:], v_sb[:, kcp, :, 2 * g2 + 1, 0 : HD + 1],
                        at2[:, :, 512:1024],
                        perf_mode=mybir.MatmulPerfMode.DoubleRow,
                        start=(kcp == 0), stop=(kcp == KC // 2 - 1),
                        skip_group_check=True,
                    )

                pend = []
                for kc in range(KC):
                    kcp, j = divmod(kc, 2)
                    sc = sc_dve[:] if kc % 3 == 2 else sc_pair[:, kc % 3, :]
                    nc.tensor.matmul(
                        sc[:, 0:512],
                        kt_sb[0:64, g2, ts(kc, 128)], qt_sb[0:64, s, :],
                        start=True, stop=True,
                    )
                    nc.tensor.matmul(
                        sc[:, 512:1024],
                        kt_sb[64:128, g2, ts(kc, 128)], qt_sb[64:128, s, :],
                        start=True, stop=True,
                    )
                    at_flat = at_big.rearrange("p r j q -> p (r j) q")
                    if kc % 3 == 2:
                        # DVE fast-exp2: offloads ~1/3 of the exp work from
                        # the saturated ScalarE onto VectorE.
                        it = itp.tile([128, 1024], I32, tag="it")
                        nc.vector.tensor_scalar(
                            out=it[:], in0=sc[:],
                            scalar1=EXP_SCALE, scalar2=EXP_OFFSET,
                            op0=mybir.AluOpType.mult,
                            op1=mybir.AluOpType.add,
                        )
                        nc.vector.tensor_copy(
                            out=at_flat[:, kc % 6, :], in_=it[:].bitcast(F32)
                        )
                    elif kc % 3 == 1:
                        # merged exp over this kc and the previous one: their
                        # score ring slots (0,1) and attn ring slots are
                        # contiguous by construction.
                        nc.scalar.activation(
                            at_flat[:, (kc - 1) % 6 : (kc - 1) % 6 + 2, :],
                            sc_pair[:, 0:2, :],
                            mybir.ActivationFunctionType.Exp, scale=0.125,
                        )
                    if j == 1:
                        pend.append(kcp)
                        if len(pend) > 2:
                            av(pend.pop(0))
                for item in pend:
                    av(item)

                # Spill ctx PSUM -> SBUF right away (ScalarE for head a,
                # VectorE for head b) so the 2-deep ctx PSUM ring frees for
                # the next slot without waiting on the normalize chain --
                # otherwise the PE idles ~7us per slot boundary and the HAM
                # clock-gate re-throttles it to 1.2 GHz.
                cs_a = csp.tile([HD + 1, QS], F32, tag="cs")
                nc.scalar.copy(cs_a[:], ctx_a[:])
                cs_b = csp.tile([HD + 1, QS], F32, tag="cs")
                nc.vector.tensor_copy(out=cs_b[:], in_=ctx_b[:])

                # normalize: ctxT[d, q] * (1/denom[q]); head a -> parts 0:64,
                # head b -> parts 64:128 (via sb2sb DMA partition shift).
                dn_a = normp.tile([1, QS], F32, tag="dn")
                nc.vector.tensor_copy(out=dn_a[:], in_=cs_a[HD : HD + 1, :])
                db_a = normp.tile([64, QS], F32, tag="db")
                nc.gpsimd.partition_broadcast(db_a[:], dn_a[:], channels=64)
                rb_a = normp.tile([64, QS], F32, tag="rbcast")
                nc.vector.reciprocal_approx_fast(rb_a[:], db_a[:])
                nc.gpsimd.tensor_mul(
                    ctxt_sb[0:64, s, :], cs_a[0:HD, :], rb_a[:]
                )

                dn_b = normp.tile([1, QS], F32, tag="dn")
                nc.vector.tensor_copy(out=dn_b[:], in_=cs_b[HD : HD + 1, :])
                db_b = normp.tile([64, QS], F32, tag="db")
                nc.gpsimd.partition_broadcast(db_b[:], dn_b[:], channels=64)
                rb_b = normp.tile([64, QS], F32, tag="rbcast")
                nc.vector.reciprocal_approx_fast(rb_b[:], db_b[:])
                tmp = oddp.tile([64, QS], F16, tag="odd")
                nc.gpsimd.tensor_mul(tmp[:], cs_b[0:HD, :], rb_b[:])
                nc.sync.dma_start(ctxt_sb[64:128, s, :], tmp[:])

            for s in range(8):
                attn_slot(s)

            # ---- output projection ----
            for qt in range(QT):
                po = sc_pair[:, qt % 2, :]
                for half in range(2):
                    nc.tensor.matmul(
                        po[:, ts(half, 512)],
                        ones_sb[0:1, 0:128], bo_sb[0:1, ts(half, 512)],
                        start=True, stop=False,
                    )
                    for s in range(8):
                        nc.tensor.matmul(
                            po[:, ts(half, 512)],
                            ctxt_sb[:, s, ts(qt, 128)],
                            wo_sb[:, s, ts(half, 512)],
                            start=False, stop=(s == 7),
                        )
                ob = outsb.tile([128, DM], F32, tag="ob")
                nc.vector.tensor_copy(out=ob[:], in_=po[:])
                nc.sync.dma_start(out[ts(qt, 128), :], ob[:])


def build():
    if "nc" in _CACHE:
        return _CACHE["nc"]
    nc = bacc.Bacc(
        "TRN2", target_bir_lowering=False, debug=False, num_devices=N_CORES
    )
    with tile.TileContext(nc) as tc:
        _emit(tc)
    nc.compile()
    _CACHE["nc"] = nc
    return nc


def make_in_maps(inputs) -> list[dict]:
    """Host-side staging: cast to fp16 and pre-shuffle into SBUF layouts."""
    x = np.asarray(inputs["x"], dtype=np.float32).reshape(SEQ, DM)
    Wq = np.asarray(inputs["Wq"], dtype=np.float32).reshape(DM, DM)
    bq = np.asarray(inputs["bq"], dtype=np.float32).reshape(DM)
    Wk = np.asarray(inputs["Wk"], dtype=np.float32).reshape(DM, KV)
    bk = np.asarray(inputs["bk"], dtype=np.float32).reshape(KV)
    Wv = np.asarray(inputs["Wv"], dtype=np.float32).reshape(DM, KV)
    bv = np.asarray(inputs["bv"], dtype=np.float32).reshape(KV)
    Wo = np.asarray(inputs["Wo"], dtype=np.float32).reshape(DM, DM)
    bo = np.asarray(inputs["bo"], dtype=np.float32).reshape(DM)

    # x^T as [p, cc, seq]
    xt16 = np.ascontiguousarray(
        x.T.reshape(CC, 128, SEQ).transpose(1, 0, 2).astype(np.float16)
    )
    # Wk/Wv as [p, cc, kv]
    wk16 = np.ascontiguousarray(
        Wk.reshape(CC, 128, KV).transpose(1, 0, 2).astype(np.float16)
    )
    wv16 = np.ascontiguousarray(
        Wv.reshape(CC, 128, KV).transpose(1, 0, 2).astype(np.float16)
    )
    # Wq shuffled: slot s = 4*g2+i holds q-head pair (8*g2+i, 8*g2+i+4);
    # model col for (s, half, d) is 512*g2 + 256*half + 64*i + d.
    Wqr = Wq.reshape(CC, 128, DM)
    wq16 = np.zeros((128, CC, DM), np.float16)
    bq16 = np.zeros((1, DM), np.float16)
    wo16 = np.zeros((128, CC, DM), np.float16)
    for g2 in range(2):
        for i in range(4):
            s = 4 * g2 + i
            for h in range(2):
                col = 512 * g2 + 256 * h + 64 * i
                dst = 128 * s + 64 * h
                wq16[:, :, dst : dst + 64] = Wqr[:, :, col : col + 64].transpose(
                    1, 0, 2
                )
                bq16[0, dst : dst + 64] = bq[col : col + 64]
                wo16[64 * h : 64 * h + 64, s, :] = Wo[col : col + 64, :]
    shared = {
        "wq": wq16,
        "bq2": np.ascontiguousarray(bq16[0].reshape(CC, 128).T.astype(np.float32)),
        "wk": wk16,
        "bk2": np.ascontiguousarray(bk.astype(np.float32).reshape(2, 128).T),
        "wv": wv16,
        "bv": bv.reshape(1, KV).astype(np.float16),
        "wo": wo16,
        "bo": bo.reshape(1, DM).astype(np.float16),
    }
    return [
        dict(
            shared,
            xqt=np.ascontiguousarray(xt16[:, :, c * QS : (c + 1) * QS]),
        )
        for c in range(N_CORES)
    ]


def kernel(**inputs) -> np.ndarray:
    nc = build()
    in_maps = make_in_maps(inputs)
    res = run_bass_kernel_spmd(nc, in_maps, core_ids=list(range(N_CORES)))
    full = np.concatenate([res.results[c]["out"] for c in range(N_CORES)], axis=0)
    return full[None].astype(np.float32)


if __name__ == "__main__":
    rng = np.random.default_rng(0)
    s = 0.02
    inputs = {
        "x": rng.standard_normal((1, SEQ, DM), dtype=np.float32),
        "Wq": rng.standard_normal((DM, DM), dtype=np.float32) * s,
        "bq": rng.standard_normal((DM,), dtype=np.float32) * s,
        "Wk": rng.standard_normal((DM, KV), dtype=np.float32) * s,
        "bk": rng.standard_normal((KV,), dtype=np.float32) * s,
        "Wv": rng.standard_normal((DM, KV), dtype=np.float32) * s,
        "bv": rng.standard_normal((KV,), dtype=np.float32) * s,
        "Wo": rng.standard_normal((DM, DM), dtype=np.float32) * s,
        "bo": rng.standard_normal((DM,), dtype=np.float32) * s,
    }
    out = kernel(**inputs)
    print("out shape", out.shape, "finite", np.isfinite(out).all())


# revision 26
# speedup vs baseline: 1.1877x; 1.1877x over previous
"""GQA attention kernel for Trainium2, 8-core sequence-parallel SPMD.

Model: d_model=1024, 16 q-heads / 4 kv-heads of dim 64, seq 4096, batch 1.

Per-core split: core c handles query rows [512c, 512c+512) for ALL 16 heads,
and (redundantly) computes the full K/V projections. No collectives needed;
the host concatenates the 8 per-core [512, 1024] outputs.

v2: all input staging (fp32->fp16 cast, x transpose, Wq/Wo head-pair
shuffles) happens on the HOST in numpy; the device receives fp16 tensors in
their final SBUF layouts and just DMA-loads them.  Emission order interleaves
the second half of the projections with the first attention slots so ScalarE
(softmax exp, the critical engine) starts as early as possible.

Layout strategy ("transposed scores"):
  - xT [dm, seq] fp16 loaded directly (host pre-transposed).
  - kT[d, seq] = Wk^T @ x^T, qT[d, q] = Wq^T @ xq^T, v[seq, d] = x @ Wv
    (ones-augmented with a 65th column for softmax denominators).
  - scoresT[k, q] = kT^T(slice) @ qT: two K=64 matmuls row-packed into the
    128x128 PE array (q-head pairs chosen cross-kv so each head's kv slice
    naturally sits in the right partition half) -> concurrent on sub-arrays.
  - exp on ScalarE straight out of PSUM (scores bounded ~|3.4|, no max pass),
    fp16 attn written to SBUF.
  - contextT[d(+sum), q] accumulated over 32 k-chunks; row 64 = softmax
    denominator. Normalize with approx-reciprocal + gpsimd broadcast + DVE.
  - out = contextT^T @ Wo + bo accumulated over 8 shuffled d-chunks.
"""

import sys
import numpy as np

sys.path.insert(0, "/opt/trn_rl_repo")

from contextlib import ExitStack  # noqa: E402

import concourse.bass as bass  # noqa: E402
import concourse.bacc as bacc  # noqa: E402
import concourse.tile as tile  # noqa: E402
from concourse import mybir  # noqa: E402
from concourse.bass_utils import run_bass_kernel_spmd  # noqa: E402

N_CORES = 8
SEQ = 4096
DM = 1024
QS = SEQ // N_CORES  # 512 query rows per core
HD = 64
NQ = 16
NKV = 4
KV = NKV * HD  # 256
CC = DM // 128  # 8 contraction chunks
KC = SEQ // 128  # 32 key chunks
QT = QS // 128  # 4 query row tiles
F16 = mybir.dt.float16
F32 = mybir.dt.float32
I32 = mybir.dt.int32
F8 = mybir.dt.float8e4
ts = bass.ts

# DVE fast-exp2: attn = bitcast_f32(int32(score*EXP_SCALE + EXP_OFFSET)).
# EXP_SCALE folds the 1/sqrt(d) softmax scale and log2(e) into the fp32
# exponent/mantissa construction; EXP_OFFSET carries the exponent bias with
# the balanced magic constant (max rel err ~3% on the affected tiles).
EXP_SCALE = float(0.125 * np.log2(np.e) * (1 << 23))
EXP_OFFSET = float((127.0 - 0.0434) * (1 << 23))

_CACHE = {}


def _emit(tc: tile.TileContext):
    nc = tc.nc
    # All inputs pre-laid-out on host, fp16.
    xqt = nc.dram_tensor("xqt", [128, CC, QS], F16, kind="ExternalInput").ap()
    Wq = nc.dram_tensor("wq", [128, CC, DM], F16, kind="ExternalInput").ap()
    bq2 = nc.dram_tensor("bq2", [128, CC], F32, kind="ExternalInput").ap()
    Wk = nc.dram_tensor("wk", [128, CC, KV], F16, kind="ExternalInput").ap()
    bk2 = nc.dram_tensor("bk2", [128, 2], F32, kind="ExternalInput").ap()
    Wv = nc.dram_tensor("wv", [128, CC, KV], F16, kind="ExternalInput").ap()
    bv = nc.dram_tensor("bv", [1, KV], F16, kind="ExternalInput").ap()
    Wo = nc.dram_tensor("wo", [128, CC, DM], F16, kind="ExternalInput").ap()
    bo = nc.dram_tensor("bo", [1, DM], F16, kind="ExternalInput").ap()
    out = nc.dram_tensor("out", [QS, DM], F32, kind="ExternalOutput").ap()

    stack = ExitStack()
    with stack:
        consts = stack.enter_context(tc.tile_pool(name="consts", bufs=1))
        # ---- weight/bias loads (already fp16, final layout) ----
        wk_sb = consts.tile([128, CC, KV], F16)
        bk_sb = consts.tile([128, 2], F32)
        wv_sb = consts.tile([128, CC, KV], F16)
        bv_sb = consts.tile([1, KV], F16)
        wq_sb = consts.tile([128, CC, DM], F16)
        bq_sb = consts.tile([128, CC], F32)
        wo_sb = consts.tile([128, CC, DM], F16)
        bo_sb = consts.tile([1, DM], F16)
        ones_sb = consts.tile([1, 512], F16)
        nc.vector.memset(ones_sb[:], 1.0)

# persistent activations
        acts = stack.enter_context(tc.tile_pool(name="acts", bufs=1))
        xqt_sb = acts.tile([128, CC, QS], F16)
        kt_sb = acts.tile([128, 2, SEQ], F16)      # kv dims (pairs) x seq
        # [kc-pair, pair-parity, kv head, d(+1, pad to 68)] fp8 for DoubleRow
        v_sb = acts.tile([128, KC // 2, 2, NKV, 68], F8)
        qt_sb = acts.tile([128, CC, QS], F16)      # shuffled q dims x q-rows
        ctxt_sb = acts.tile([128, CC, QS], F16)
        kt_loc = acts.tile([128, 2, QS], F16)
        v_loc = acts.tile([128, 2, 4, 2, 68], F8)  # [gpair, m, g, d+1pad]
        nc.gpsimd.memset(v_loc[:, :, :, :, HD], 1.0)

        # DMA priority: sync queue carries the k/v local projection critical
        # path, gpsimd queue the q path, scalar queue the (late-needed)
        # out-proj weights.
        nc.sync.dma_start(wk_sb[:], Wk)
        nc.sync.dma_start(bk_sb[:], bk2)
        nc.sync.dma_start(xqt_sb[:], xqt)
        nc.sync.dma_start(wv_sb[:], Wv)
        nc.sync.dma_start(bv_sb[:], bv)
        nc.sync.dma_start(bq_sb[:], bq2)
        for cc in range(CC):
            nc.sync.dma_start(wq_sb[:, cc, :], Wq[:, cc, :])
        for cc in range(CC):
            nc.scalar.dma_start(wo_sb[:, cc, :], Wo[:, cc, :])
        nc.scalar.dma_start(bo_sb[:], bo)

        # ---- phase 1: local projections + k/v AllGather ----
        # Each core projects only its own 512 rows of x; kT/v slices for the
        # other 7/8 of the sequence come from an on-chip AllGather instead of
        # being recomputed 8x.
        dramp = stack.enter_context(tc.tile_pool(name="dram", bufs=1, space="DRAM"))
        CCW = QS + 4 * 2 * 68 // 2  # kt pair slice + 2 v heads (fp8 packed)
        cc_in1 = dramp.tile([128, CCW], F16)
        cc_in2 = dramp.tile([128, CCW], F16)
        cc_out1 = dramp.tile([N_CORES, 128, CCW], F16)
        cc_out2 = dramp.tile([N_CORES, 128, CCW], F16)

        with tc.tile_pool(name="proj_ps", bufs=2, space="PSUM") as projp:
            # local kT slice: [128 dims of kv-head pair (2j, 2j+1), own 512]
            def kproj_loc(j):
                ps = projp.tile([128, 512], F32, tag="proj")
                for cc in range(CC):
                    nc.tensor.matmul(
                        ps[:], wk_sb[:, cc, ts(j, 128)], xqt_sb[:, cc, :],
                        start=(cc == 0), stop=(cc == CC - 1),
                    )
                nc.vector.tensor_scalar(
                    out=kt_loc[:, j, :], in0=ps[:],
                    scalar1=bk_sb[:, j : j + 1], scalar2=None,
                    op0=mybir.AluOpType.add,
                )

            # local v slice: 4 chunks of [128 rows, 4 kv heads x 64] + ones
            def vproj_loc(m):
                ps = projp.tile([128, 512], F32, tag="proj")
                nc.tensor.matmul(
                    ps[:, 0:KV], ones_sb[0:1, 0:128], bv_sb[0:1, :],
                    start=True, stop=False,
                )
                for cc in range(CC):
                    nc.tensor.matmul(
                        ps[:, 0:KV], xqt_sb[:, cc, ts(m, 128)], wv_sb[:, cc, :],
                        start=False, stop=(cc == CC - 1),
                    )
                nc.vector.tensor_copy(
                    out=v_loc[:, :, m, :, 0:HD],
                    in_=ps[:, 0:KV].rearrange("p (gp g d) -> p gp g d", gp=2, g=2),
                )

            kproj_loc(0)
            for m in range(4):
                vproj_loc(m)
            nc.sync.dma_start(cc_in1[:, 0:QS], kt_loc[:, 0, :])
            nc.sync.dma_start(
                cc_in1[:, QS:],
                v_loc[:, 0].rearrange("p m g d -> p (m g d)").bitcast(F16),
            )
            nc.gpsimd.collective_compute(
                "AllGather",
                mybir.AluOpType.bypass,
                replica_groups=[list(range(N_CORES))],
                ins=[cc_in1.opt()],
                outs=[cc_out1.opt()],
            )
            for c in range(N_CORES):
                nc.scalar.dma_start(kt_sb[:, 0, ts(c, QS)], cc_out1[c, :, 0:QS])
                nc.scalar.dma_start(
                    v_sb[:, 2 * c : 2 * c + 2, :, 0:2, :].bitcast(F16),
                    cc_out1[c, :, QS:].rearrange(
                        "p (k j g e) -> p k j g e", k=2, j=2, g=2
                    ),
                )

            # q projection overlaps the first collective
            def qproj(s):
                ps = projp.tile([128, 512], F32, tag="proj")
                for cc in range(CC):
                    nc.tensor.matmul(
                        ps[:], wq_sb[:, cc, ts(s, 128)], xqt_sb[:, cc, :],
                        start=(cc == 0), stop=(cc == CC - 1),
                    )
                nc.vector.tensor_scalar(
                    out=qt_sb[:, s, :], in0=ps[:],
                    scalar1=bq_sb[:, s : s + 1], scalar2=None,
                    op0=mybir.AluOpType.add,
                )

            for s in range(8):
                qproj(s)

            kproj_loc(1)
            nc.sync.dma_start(cc_in2[:, 0:QS], kt_loc[:, 1, :])
            nc.sync.dma_start(
                cc_in2[:, QS:],
                v_loc[:, 1].rearrange("p m g d -> p (m g d)").bitcast(F16),
            )
            nc.gpsimd.collective_compute(
                "AllGather",
                mybir.AluOpType.bypass,
                replica_groups=[list(range(N_CORES))],
                ins=[cc_in2.opt()],
                outs=[cc_out2.opt()],
            )
            for c in range(N_CORES):
                nc.scalar.dma_start(kt_sb[:, 1, ts(c, QS)], cc_out2[c, :, 0:QS])
                nc.scalar.dma_start(
                    v_sb[:, 2 * c : 2 * c + 2, :, 2:4, :].bitcast(F16),
                    cc_out2[c, :, QS:].rearrange(
                        "p (k j g e) -> p k j g e", k=2, j=2, g=2
                    ),
                )

        # ---- phase 2: attention (scores triple-buffered, AV lags 2 kc) ----
        with (
            tc.tile_pool(name="scores_ps", bufs=3, space="PSUM") as scoresp,
            tc.tile_pool(name="ctx_ps", bufs=2, space="PSUM") as ctxp,
            tc.tile_pool(name="attn", bufs=5) as attnp,
            tc.tile_pool(name="it", bufs=2) as itp,
            tc.tile_pool(name="norm", bufs=2) as normp,
            tc.tile_pool(name="cs", bufs=4) as csp,
            tc.tile_pool(name="odd", bufs=2) as oddp,
            tc.tile_pool(name="out_sb", bufs=2) as outsb,
        ):
            def attn_slot(s):
                g2, _i = divmod(s, 4)
                ctx_a = ctxp.tile([HD + 1, QS], F32, tag="ctx")
                ctx_b = ctxp.tile([HD + 1, QS], F32, tag="ctx")

                def av(kcp, at2):
                    nc.tensor.matmul(
                        ctx_a[:], v_sb[:, kcp, :, 2 * g2, 0 : HD + 1],
                        at2[:, :, 0:512],
                        perf_mode=mybir.MatmulPerfMode.DoubleRow,
                        start=(kcp == 0), stop=(kcp == KC // 2 - 1),
                        skip_group_check=True,
                    )
                    nc.tensor.matmul(
                        ctx_b[:], v_sb[:, kcp, :, 2 * g2 + 1, 0 : HD + 1],
                        at2[:, :, 512:1024],
                        perf_mode=mybir.MatmulPerfMode.DoubleRow,
                        start=(kcp == 0), stop=(kcp == KC // 2 - 1),
                        skip_group_check=True,
                    )

                pend = []
                for kcp in range(KC // 2):
                    at2 = attnp.tile([128, 2, 1024], F8, tag="at")
                    for j in range(2):
                        kc = 2 * kcp + j
                        sc = scoresp.tile([128, 1024], F32, tag="sc")
                        nc.tensor.matmul(
                            sc[:, 0:512],
                            kt_sb[0:64, g2, ts(kc, 128)], qt_sb[0:64, s, :],
                            start=True, stop=True,
                        )
                        nc.tensor.matmul(
                            sc[:, 512:1024],
                            kt_sb[64:128, g2, ts(kc, 128)], qt_sb[64:128, s, :],
                            start=True, stop=True,
                        )
                        if kc % 3 == 2:
                            # DVE fast-exp2: offloads ~1/3 of the exp work
                            # from the saturated ScalarE onto VectorE.
                            it = itp.tile([128, 1024], I32, tag="it")
                            nc.vector.tensor_scalar(
                                out=it[:], in0=sc[:],
                                scalar1=EXP_SCALE, scalar2=EXP_OFFSET,
                                op0=mybir.AluOpType.mult,
                                op1=mybir.AluOpType.add,
                            )
                            nc.vector.tensor_copy(
                                out=at2[:, j, :], in_=it[:].bitcast(F32)
                            )
                        else:
                            nc.scalar.activation(
                                at2[:, j, :], sc[:],
                                mybir.ActivationFunctionType.Exp, scale=0.125,
                            )
                    pend.append((kcp, at2))
                    if len(pend) > 2:
                        av(*pend.pop(0))
                for item in pend:
                    av(*item)

                # Spill ctx PSUM -> SBUF right away (ScalarE for head a,
                # VectorE for head b) so the 2-deep ctx PSUM ring frees for
                # the next slot without waiting on the normalize chain --
                # otherwise the PE idles ~7us per slot boundary and the HAM
                # clock-gate re-throttles it to 1.2 GHz.
                cs_a = csp.tile([HD + 1, QS], F32, tag="cs")
                nc.scalar.copy(cs_a[:], ctx_a[:])
                cs_b = csp.tile([HD + 1, QS], F32, tag="cs")
                nc.vector.tensor_copy(out=cs_b[:], in_=ctx_b[:])

                # normalize: ctxT[d, q] * (1/denom[q]); head a -> parts 0:64,
                # head b -> parts 64:128 (via sb2sb DMA partition shift).
                dn_a = normp.tile([1, QS], F32, tag="dn")
                nc.vector.tensor_copy(out=dn_a[:], in_=cs_a[HD : HD + 1, :])
                db_a = normp.tile([64, QS], F32, tag="db")
                nc.gpsimd.partition_broadcast(db_a[:], dn_a[:], channels=64)
                rb_a = normp.tile([64, QS], F32, tag="rbcast")
                nc.vector.reciprocal_approx_fast(rb_a[:], db_a[:])
                nc.gpsimd.tensor_mul(
                    ctxt_sb[0:64, s, :], cs_a[0:HD, :], rb_a[:]
                )

                dn_b = normp.tile([1, QS], F32, tag="dn")
                nc.vector.tensor_copy(out=dn_b[:], in_=cs_b[HD : HD + 1, :])
                db_b = normp.tile([64, QS], F32, tag="db")
                nc.gpsimd.partition_broadcast(db_b[:], dn_b[:], channels=64)
                rb_b = normp.tile([64, QS], F32, tag="rbcast")
                nc.vector.reciprocal_approx_fast(rb_b[:], db_b[:])
                tmp = oddp.tile([64, QS], F16, tag="odd")
                nc.gpsimd.tensor_mul(tmp[:], cs_b[0:HD, :], rb_b[:])
                nc.sync.dma_start(ctxt_sb[64:128, s, :], tmp[:])

            for s in range(8):
                attn_slot(s)

            # ---- output projection ----
            for qt in range(QT):
                po = scoresp.tile([128, 1024], F32, tag="sc")
                for half in range(2):
                    nc.tensor.matmul(
                        po[:, ts(half, 512)],
                        ones_sb[0:1, 0:128], bo_sb[0:1, ts(half, 512)],
                        start=True, stop=False,
                    )
                    for s in range(8):
                        nc.tensor.matmul(
                            po[:, ts(half, 512)],
                            ctxt_sb[:, s, ts(qt, 128)],
                            wo_sb[:, s, ts(half, 512)],
                            start=False, stop=(s == 7),
                        )
                ob = outsb.tile([128, DM], F32, tag="ob")
                nc.vector.tensor_copy(out=ob[:], in_=po[:])
                nc.sync.dma_start(out[ts(qt, 128), :], ob[:])


def build():
    if "nc" in _CACHE:
        return _CACHE["nc"]
    nc = bacc.Bacc(
        "TRN2", target_bir_lowering=False, debug=False, num_devices=N_CORES
    )
    with tile.TileContext(nc) as tc:
        _emit(tc)
    nc.compile()
    _CACHE["nc"] = nc
    return nc


def make_in_maps(inputs) -> list[dict]:
    """Host-side staging: cast to fp16 and pre-shuffle into SBUF layouts."""
    x = np.asarray(inputs["x"], dtype=np.float32).reshape(SEQ, DM)
    Wq = np.asarray(inputs["Wq"], dtype=np.float32).reshape(DM, DM)
    bq = np.asarray(inputs["bq"], dtype=np.float32).reshape(DM)
    Wk = np.asarray(inputs["Wk"], dtype=np.float32).reshape(DM, KV)
    bk = np.asarray(inputs["bk"], dtype=np.float32).reshape(KV)
    Wv = np.asarray(inputs["Wv"], dtype=np.float32).reshape(DM, KV)
    bv = np.asarray(inputs["bv"], dtype=np.float32).reshape(KV)
    Wo = np.asarray(inputs["Wo"], dtype=np.float32).reshape(DM, DM)
    bo = np.asarray(inputs["bo"], dtype=np.float32).reshape(DM)

    # x^T as [p, cc, seq]
    xt16 = np.ascontiguousarray(
        x.T.reshape(CC, 128, SEQ).transpose(1, 0, 2).astype(np.float16)
    )
    # Wk/Wv as [p, cc, kv]
    wk16 = np.ascontiguousarray(
        Wk.reshape(CC, 128, KV).transpose(1, 0, 2).astype(np.float16)
    )
    wv16 = np.ascontiguousarray(
        Wv.reshape(CC, 128, KV).transpose(1, 0, 2).astype(np.float16)
    )
    # Wq shuffled: slot s = 4*g2+i holds q-head pair (8*g2+i, 8*g2+i+4);
    # model col for (s, half, d) is 512*g2 + 256*half + 64*i + d.
    Wqr = Wq.reshape(CC, 128, DM)
    wq16 = np.zeros((128, CC, DM), np.float16)
    bq16 = np.zeros((1, DM), np.float16)
    wo16 = np.zeros((128, CC, DM), np.float16)
    for g2 in range(2):
        for i in range(4):
            s = 4 * g2 + i
            for h in range(2):
                col = 512 * g2 + 256 * h + 64 * i
                dst = 128 * s + 64 * h
                wq16[:, :, dst : dst + 64] = Wqr[:, :, col : col + 64].transpose(
                    1, 0, 2
                )
                bq16[0, dst : dst + 64] = bq[col : col + 64]
                wo16[64 * h : 64 * h + 64, s, :] = Wo[col : col + 64, :]
    shared = {
        "wq": wq16,
        "bq2": np.ascontiguousarray(bq16[0].reshape(CC, 128).T.astype(np.float32)),
        "wk": wk16,
        "bk2": np.ascontiguousarray(bk.astype(np.float32).reshape(2, 128).T),
        "wv": wv16,
        "bv": bv.reshape(1, KV).astype(np.float16),
        "wo": wo16,
        "bo": bo.reshape(1, DM).astype(np.float16),
    }
    return [
        dict(
            shared,
            xqt=np.ascontiguousarray(xt16[:, :, c * QS : (c + 1) * QS]),
        )
        for c in range(N_CORES)
    ]


def kernel(**inputs) -> np.ndarray:
    nc = build()
    in_maps = make_in_maps(inputs)
    res = run_bass_kernel_spmd(nc, in_maps, core_ids=list(range(N_CORES)))
    full = np.concatenate([res.results[c]["out"] for c in range(N_CORES)], axis=0)
    return full[None].astype(np.float32)


if __name__ == "__main__":
    rng = np.random.default_rng(0)
    s = 0.02
    inputs = {
        "x": rng.standard_normal((1, SEQ, DM), dtype=np.float32),
        "Wq": rng.standard_normal((DM, DM), dtype=np.float32) * s,
        "bq": rng.standard_normal((DM,), dtype=np.float32) * s,
        "Wk": rng.standard_normal((DM, KV), dtype=np.float32) * s,
        "bk": rng.standard_normal((KV,), dtype=np.float32) * s,
        "Wv": rng.standard_normal((DM, KV), dtype=np.float32) * s,
        "bv": rng.standard_normal((KV,), dtype=np.float32) * s,
        "Wo": rng.standard_normal((DM, DM), dtype=np.float32) * s,
        "bo": rng.standard_normal((DM,), dtype=np.float32) * s,
    }
    out = kernel(**inputs)
    print("out shape", out.shape, "finite", np.isfinite(out).all())


# revision 27
# speedup vs baseline: 1.3473x; 1.1344x over previous
"""GQA attention kernel for Trainium2, 8-core sequence-parallel SPMD.

Model: d_model=1024, 16 q-heads / 4 kv-heads of dim 64, seq 4096, batch 1.

Per-core split: core c handles query rows [512c, 512c+512) for ALL 16 heads,
and (redundantly) computes the full K/V projections. No collectives needed;
the host concatenates the 8 per-core [512, 1024] outputs.

v2: all input staging (fp32->fp16 cast, x transpose, Wq/Wo head-pair
shuffles) happens on the HOST in numpy; the device receives fp16 tensors in
their final SBUF layouts and just DMA-loads them.  Emission order interleaves
the second half of the projections with the first attention slots so ScalarE
(softmax exp, the critical engine) starts as early as possible.

Layout strategy ("transposed scores"):
  - xT [dm, seq] fp16 loaded directly (host pre-transposed).
  - kT[d, seq] = Wk^T @ x^T, qT[d, q] = Wq^T @ xq^T, v[seq, d] = x @ Wv
    (ones-augmented with a 65th column for softmax denominators).
  - scoresT[k, q] = kT^T(slice) @ qT: two K=64 matmuls row-packed into the
    128x128 PE array (q-head pairs chosen cross-kv so each head's kv slice
    naturally sits in the right partition half) -> concurrent on sub-arrays.
  - exp on ScalarE straight out of PSUM (scores bounded ~|3.4|, no max pass),
    fp16 attn written to SBUF.
  - contextT[d(+sum), q] accumulated over 32 k-chunks; row 64 = softmax
    denominator. Normalize with approx-reciprocal + gpsimd broadcast + DVE.
  - out = contextT^T @ Wo + bo accumulated over 8 shuffled d-chunks.
"""

import sys
import numpy as np

sys.path.insert(0, "/opt/trn_rl_repo")

from contextlib import ExitStack  # noqa: E402

import concourse.bass as bass  # noqa: E402
import concourse.bacc as bacc  # noqa: E402
import concourse.tile as tile  # noqa: E402
from concourse import mybir  # noqa: E402
from concourse.bass_utils import run_bass_kernel_spmd  # noqa: E402

N_CORES = 8
SEQ = 4096
DM = 1024
QS = SEQ // N_CORES  # 512 query rows per core
HD = 64
NQ = 16
NKV = 4
KV = NKV * HD  # 256
CC = DM // 128  # 8 contraction chunks
KC = SEQ // 128  # 32 key chunks
QT = QS // 128  # 4 query row tiles
F16 = mybir.dt.float16
F32 = mybir.dt.float32
I32 = mybir.dt.int32
F8 = mybir.dt.float8e4
ts = bass.ts

# DVE fast-exp2: attn = bitcast_f32(int32(score*EXP_SCALE + EXP_OFFSET)).
# EXP_SCALE folds the 1/sqrt(d) softmax scale and log2(e) into the fp32
# exponent/mantissa construction; EXP_OFFSET carries the exponent bias with
# the balanced magic constant (max rel err ~3% on the affected tiles).
EXP_SCALE = float(0.125 * np.log2(np.e) * (1 << 23))
EXP_OFFSET = float((127.0 - 0.0434) * (1 << 23))

_CACHE = {}


def _emit(tc: tile.TileContext):
    nc = tc.nc
    # All inputs pre-laid-out on host, fp16.
    xqt = nc.dram_tensor("xqt", [128, CC, QS], F16, kind="ExternalInput").ap()
    Wq = nc.dram_tensor("wq", [128, CC, DM], F16, kind="ExternalInput").ap()
    bq2 = nc.dram_tensor("bq2", [128, CC], F32, kind="ExternalInput").ap()
    Wk = nc.dram_tensor("wk", [128, CC, KV], F16, kind="ExternalInput").ap()
    bk2 = nc.dram_tensor("bk2", [128, 2], F32, kind="ExternalInput").ap()
    Wv = nc.dram_tensor("wv", [128, CC, KV], F16, kind="ExternalInput").ap()
    bv = nc.dram_tensor("bv", [1, KV], F16, kind="ExternalInput").ap()
    Wo = nc.dram_tensor("wo", [128, CC, DM], F16, kind="ExternalInput").ap()
    bo = nc.dram_tensor("bo", [1, DM], F16, kind="ExternalInput").ap()
    out = nc.dram_tensor("out", [QS, DM], F32, kind="ExternalOutput").ap()

    stack = ExitStack()
    with stack:
        consts = stack.enter_context(tc.tile_pool(name="consts", bufs=1))
        # ---- weight/bias loads (already fp16, final layout) ----
        wk_sb = consts.tile([128, CC, KV], F16)
        bk_sb = consts.tile([128, 2], F32)
        wv_sb = consts.tile([128, CC, KV], F16)
        bv_sb = consts.tile([1, KV], F16)
        wq_sb = consts.tile([128, CC, DM], F16)
        bq_sb = consts.tile([128, CC], F32)
        wo_sb = consts.tile([128, CC, DM], F16)
        bo_sb = consts.tile([1, DM], F16)
        ones_sb = consts.tile([1, 512], F16)
        nc.vector.memset(ones_sb[:], 1.0)

# persistent activations
        acts = stack.enter_context(tc.tile_pool(name="acts", bufs=1))
        xqt_sb = acts.tile([128, CC, QS], F16)
        kt_sb = acts.tile([128, 2, SEQ], F16)      # kv dims (pairs) x seq
        # [kc-pair, pair-parity, kv head, d(+1, pad to 68)] fp8 for DoubleRow
        v_sb = acts.tile([128, KC // 2, 2, NKV, 68], F8)
        qt_sb = acts.tile([128, CC, QS], F16)      # shuffled q dims x q-rows
        ctxt_sb = acts.tile([128, CC, QS], F16)
        kt_loc = acts.tile([128, 2, QS], F16)
        v_loc = acts.tile([128, 2, 4, 2, 68], F8)  # [gpair, m, g, d+1pad]
        nc.gpsimd.memset(v_loc[:, :, :, :, HD], 1.0)

        # DMA priority: sync queue carries the k/v local projection critical
        # path, gpsimd queue the q path, scalar queue the (late-needed)
        # out-proj weights.
        nc.sync.dma_start(wk_sb[:], Wk)
        nc.sync.dma_start(bk_sb[:], bk2)
        nc.sync.dma_start(xqt_sb[:], xqt)
        nc.sync.dma_start(wv_sb[:], Wv)
        nc.sync.dma_start(bv_sb[:], bv)
        nc.scalar.dma_start(bq_sb[:], bq2)
        for cc in range(CC):
            nc.scalar.dma_start(wq_sb[:, cc, :], Wq[:, cc, :])
        for cc in range(CC):
            nc.scalar.dma_start(wo_sb[:, cc, :], Wo[:, cc, :])
        nc.scalar.dma_start(bo_sb[:], bo)

        # ---- phase 1: local projections + k/v AllGather ----
        # Each core projects only its own 512 rows of x; kT/v slices for the
        # other 7/8 of the sequence come from an on-chip AllGather instead of
        # being recomputed 8x.
        dramp = stack.enter_context(tc.tile_pool(name="dram", bufs=1, space="DRAM"))
        CCW = QS + 4 * 2 * 68 // 2  # kt pair slice + 2 v heads (fp8 packed)
        cc_in1 = dramp.tile([128, CCW], F16)
        cc_in2 = dramp.tile([128, CCW], F16)
        cc_out1 = dramp.tile([N_CORES, 128, CCW], F16)
        cc_out2 = dramp.tile([N_CORES, 128, CCW], F16)

        with tc.tile_pool(name="proj_ps", bufs=2, space="PSUM") as projp:
            # local kT slice: [128 dims of kv-head pair (2j, 2j+1), own 512]
            def kproj_loc(j):
                ps = projp.tile([128, 512], F32, tag="proj")
                for cc in range(CC):
                    nc.tensor.matmul(
                        ps[:], wk_sb[:, cc, ts(j, 128)], xqt_sb[:, cc, :],
                        start=(cc == 0), stop=(cc == CC - 1),
                    )
                nc.vector.tensor_scalar(
                    out=kt_loc[:, j, :], in0=ps[:],
                    scalar1=bk_sb[:, j : j + 1], scalar2=None,
                    op0=mybir.AluOpType.add,
                )

            # local v slice: 4 chunks of [128 rows, 4 kv heads x 64] + ones
            def vproj_loc(m):
                ps = projp.tile([128, 512], F32, tag="proj")
                nc.tensor.matmul(
                    ps[:, 0:KV], ones_sb[0:1, 0:128], bv_sb[0:1, :],
                    start=True, stop=False,
                )
                for cc in range(CC):
                    nc.tensor.matmul(
                        ps[:, 0:KV], xqt_sb[:, cc, ts(m, 128)], wv_sb[:, cc, :],
                        start=False, stop=(cc == CC - 1),
                    )
                nc.vector.tensor_copy(
                    out=v_loc[:, :, m, :, 0:HD],
                    in_=ps[:, 0:KV].rearrange("p (gp g d) -> p gp g d", gp=2, g=2),
                )

            kproj_loc(0)
            for m in range(4):
                vproj_loc(m)
            nc.sync.dma_start(cc_in1[:, 0:QS], kt_loc[:, 0, :])
            nc.sync.dma_start(
                cc_in1[:, QS:],
                v_loc[:, 0].rearrange("p m g d -> p (m g d)").bitcast(F16),
            )
            nc.gpsimd.collective_compute(
                "AllGather",
                mybir.AluOpType.bypass,
                replica_groups=[list(range(N_CORES))],
                ins=[cc_in1.opt()],
                outs=[cc_out1.opt()],
            )
            for c in range(N_CORES):
                nc.scalar.dma_start(kt_sb[:, 0, ts(c, QS)], cc_out1[c, :, 0:QS])
                nc.scalar.dma_start(
                    v_sb[:, 2 * c : 2 * c + 2, :, 0:2, :].bitcast(F16),
                    cc_out1[c, :, QS:].rearrange(
                        "p (k j g e) -> p k j g e", k=2, j=2, g=2
                    ),
                )

            # q projection overlaps the first collective
            def qproj(s):
                ps = projp.tile([128, 512], F32, tag="proj")
                for cc in range(CC):
                    nc.tensor.matmul(
                        ps[:], wq_sb[:, cc, ts(s, 128)], xqt_sb[:, cc, :],
                        start=(cc == 0), stop=(cc == CC - 1),
                    )
                nc.vector.tensor_scalar(
                    out=qt_sb[:, s, :], in0=ps[:],
                    scalar1=bq_sb[:, s : s + 1], scalar2=None,
                    op0=mybir.AluOpType.add,
                )

            for s in range(8):
                qproj(s)

            kproj_loc(1)
            nc.sync.dma_start(cc_in2[:, 0:QS], kt_loc[:, 1, :])
            nc.sync.dma_start(
                cc_in2[:, QS:],
                v_loc[:, 1].rearrange("p m g d -> p (m g d)").bitcast(F16),
            )
            nc.gpsimd.collective_compute(
                "AllGather",
                mybir.AluOpType.bypass,
                replica_groups=[list(range(N_CORES))],
                ins=[cc_in2.opt()],
                outs=[cc_out2.opt()],
            )
            for c in range(N_CORES):
                nc.scalar.dma_start(kt_sb[:, 1, ts(c, QS)], cc_out2[c, :, 0:QS])
                nc.scalar.dma_start(
                    v_sb[:, 2 * c : 2 * c + 2, :, 2:4, :].bitcast(F16),
                    cc_out2[c, :, QS:].rearrange(
                        "p (k j g e) -> p k j g e", k=2, j=2, g=2
                    ),
                )

        # ---- phase 2: attention (scores triple-buffered, AV lags 2 kc) ----
        with (
            tc.tile_pool(name="scores_ps", bufs=3, space="PSUM") as scoresp,
            tc.tile_pool(name="ctx_ps", bufs=2, space="PSUM") as ctxp,
            tc.tile_pool(name="attn", bufs=5) as attnp,
            tc.tile_pool(name="it", bufs=2) as itp,
            tc.tile_pool(name="norm", bufs=2) as normp,
            tc.tile_pool(name="cs", bufs=4) as csp,
            tc.tile_pool(name="odd", bufs=2) as oddp,
            tc.tile_pool(name="out_sb", bufs=2) as outsb,
        ):
            def attn_slot(s):
                g2, _i = divmod(s, 4)
                ctx_a = ctxp.tile([HD + 1, QS], F32, tag="ctx")
                ctx_b = ctxp.tile([HD + 1, QS], F32, tag="ctx")

                def av(kcp, at2):
                    nc.tensor.matmul(
                        ctx_a[:], v_sb[:, kcp, :, 2 * g2, 0 : HD + 1],
                        at2[:, :, 0:512],
                        perf_mode=mybir.MatmulPerfMode.DoubleRow,
                        start=(kcp == 0), stop=(kcp == KC // 2 - 1),
                        skip_group_check=True,
                    )
                    nc.tensor.matmul(
                        ctx_b[:], v_sb[:, kcp, :, 2 * g2 + 1, 0 : HD + 1],
                        at2[:, :, 512:1024],
                        perf_mode=mybir.MatmulPerfMode.DoubleRow,
                        start=(kcp == 0), stop=(kcp == KC // 2 - 1),
                        skip_group_check=True,
                    )

                pend = []
                for kcp in range(KC // 2):
                    at2 = attnp.tile([128, 2, 1024], F8, tag="at")
                    for j in range(2):
                        kc = 2 * kcp + j
                        sc = scoresp.tile([128, 1024], F32, tag="sc")
                        nc.tensor.matmul(
                            sc[:, 0:512],
                            kt_sb[0:64, g2, ts(kc, 128)], qt_sb[0:64, s, :],
                            start=True, stop=True,
                        )
                        nc.tensor.matmul(
                            sc[:, 512:1024],
                            kt_sb[64:128, g2, ts(kc, 128)], qt_sb[64:128, s, :],
                            start=True, stop=True,
                        )
                        if kc % 3 == 2:
                            # DVE fast-exp2: offloads ~1/3 of the exp work
                            # from the saturated ScalarE onto VectorE.
                            it = itp.tile([128, 1024], I32, tag="it")
                            nc.vector.tensor_scalar(
                                out=it[:], in0=sc[:],
                                scalar1=EXP_SCALE, scalar2=EXP_OFFSET,
                                op0=mybir.AluOpType.mult,
                                op1=mybir.AluOpType.add,
                            )
                            nc.vector.tensor_copy(
                                out=at2[:, j, :], in_=it[:].bitcast(F32)
                            )
                        else:
                            nc.scalar.activation(
                                at2[:, j, :], sc[:],
                                mybir.ActivationFunctionType.Exp, scale=0.125,
                            )
                    pend.append((kcp, at2))
                    if len(pend) > 2:
                        av(*pend.pop(0))
                for item in pend:
                    av(*item)

                # Spill ctx PSUM -> SBUF right away (ScalarE for head a,
                # VectorE for head b) so the 2-deep ctx PSUM ring frees for
                # the next slot without waiting on the normalize chain --
                # otherwise the PE idles ~7us per slot boundary and the HAM
                # clock-gate re-throttles it to 1.2 GHz.
                cs_a = csp.tile([HD + 1, QS], F32, tag="cs")
                nc.scalar.copy(cs_a[:], ctx_a[:])
                cs_b = csp.tile([HD + 1, QS], F32, tag="cs")
                nc.vector.tensor_copy(out=cs_b[:], in_=ctx_b[:])

                # normalize: ctxT[d, q] * (1/denom[q]); head a -> parts 0:64,
                # head b -> parts 64:128 (via sb2sb DMA partition shift).
                dn_a = normp.tile([1, QS], F32, tag="dn")
                nc.vector.tensor_copy(out=dn_a[:], in_=cs_a[HD : HD + 1, :])
                db_a = normp.tile([64, QS], F32, tag="db")
                nc.gpsimd.partition_broadcast(db_a[:], dn_a[:], channels=64)
                rb_a = normp.tile([64, QS], F32, tag="rbcast")
                nc.vector.reciprocal_approx_fast(rb_a[:], db_a[:])
                nc.vector.tensor_mul(
                    ctxt_sb[0:64, s, :], cs_a[0:HD, :], rb_a[:]
                )

                dn_b = normp.tile([1, QS], F32, tag="dn")
                nc.vector.tensor_copy(out=dn_b[:], in_=cs_b[HD : HD + 1, :])
                db_b = normp.tile([64, QS], F32, tag="db")
                nc.gpsimd.partition_broadcast(db_b[:], dn_b[:], channels=64)
                rb_b = normp.tile([64, QS], F32, tag="rbcast")
                nc.vector.reciprocal_approx_fast(rb_b[:], db_b[:])
                tmp = oddp.tile([64, QS], F16, tag="odd")
                nc.vector.tensor_mul(tmp[:], cs_b[0:HD, :], rb_b[:])
                nc.sync.dma_start(ctxt_sb[64:128, s, :], tmp[:])

            for s in range(8):
                attn_slot(s)

            # ---- output projection ----
            for qt in range(QT):
                po = scoresp.tile([128, 1024], F32, tag="sc")
                for half in range(2):
                    nc.tensor.matmul(
                        po[:, ts(half, 512)],
                        ones_sb[0:1, 0:128], bo_sb[0:1, ts(half, 512)],
                        start=True, stop=False,
                    )
                    for s in range(8):
                        nc.tensor.matmul(
                            po[:, ts(half, 512)],
                            ctxt_sb[:, s, ts(qt, 128)],
                            wo_sb[:, s, ts(half, 512)],
                            start=False, stop=(s == 7),
                        )
                ob = outsb.tile([128, DM], F32, tag="ob")
                nc.vector.tensor_copy(out=ob[:], in_=po[:])
                nc.sync.dma_start(out[ts(qt, 128), :], ob[:])


def build():
    if "nc" in _CACHE:
        return _CACHE["nc"]
    nc = bacc.Bacc(
        "TRN2", target_bir_lowering=False, debug=False, num_devices=N_CORES
    )
    with tile.TileContext(nc) as tc:
        _emit(tc)
    nc.compile()
    _CACHE["nc"] = nc
    return nc


def make_in_maps(inputs) -> list[dict]:
    """Host-side staging: cast to fp16 and pre-shuffle into SBUF layouts."""
    x = np.asarray(inputs["x"], dtype=np.float32).reshape(SEQ, DM)
    Wq = np.asarray(inputs["Wq"], dtype=np.float32).reshape(DM, DM)
    bq = np.asarray(inputs["bq"], dtype=np.float32).reshape(DM)
    Wk = np.asarray(inputs["Wk"], dtype=np.float32).reshape(DM, KV)
    bk = np.asarray(inputs["bk"], dtype=np.float32).reshape(KV)
    Wv = np.asarray(inputs["Wv"], dtype=np.float32).reshape(DM, KV)
    bv = np.asarray(inputs["bv"], dtype=np.float32).reshape(KV)
    Wo = np.asarray(inputs["Wo"], dtype=np.float32).reshape(DM, DM)
    bo = np.asarray(inputs["bo"], dtype=np.float32).reshape(DM)

    # x^T as [p, cc, seq]
    xt16 = np.ascontiguousarray(
        x.T.reshape(CC, 128, SEQ).transpose(1, 0, 2).astype(np.float16)
    )
    # Wk/Wv as [p, cc, kv]
    wk16 = np.ascontiguousarray(
        Wk.reshape(CC, 128, KV).transpose(1, 0, 2).astype(np.float16)
    )
    wv16 = np.ascontiguousarray(
        Wv.reshape(CC, 128, KV).transpose(1, 0, 2).astype(np.float16)
    )
    # Wq shuffled: slot s = 4*g2+i holds q-head pair (8*g2+i, 8*g2+i+4);
    # model col for (s, half, d) is 512*g2 + 256*half + 64*i + d.
    Wqr = Wq.reshape(CC, 128, DM)
    wq16 = np.zeros((128, CC, DM), np.float16)
    bq16 = np.zeros((1, DM), np.float16)
    wo16 = np.zeros((128, CC, DM), np.float16)
    for g2 in range(2):
        for i in range(4):
            s = 4 * g2 + i
            for h in range(2):
                col = 512 * g2 + 256 * h + 64 * i
                dst = 128 * s + 64 * h
                wq16[:, :, dst : dst + 64] = Wqr[:, :, col : col + 64].transpose(
                    1, 0, 2
                )
                bq16[0, dst : dst + 64] = bq[col : col + 64]
                wo16[64 * h : 64 * h + 64, s, :] = Wo[col : col + 64, :]
    shared = {
        "wq": wq16,
        "bq2": np.ascontiguousarray(bq16[0].reshape(CC, 128).T.astype(np.float32)),
        "wk": wk16,
        "bk2": np.ascontiguousarray(bk.astype(np.float32).reshape(2, 128).T),
        "wv": wv16,
        "bv": bv.reshape(1, KV).astype(np.float16),
        "wo": wo16,
        "bo": bo.reshape(1, DM).astype(np.float16),
    }
    return [
        dict(
            shared,
            xqt=np.ascontiguousarray(xt16[:, :, c * QS : (c + 1) * QS]),
        )
        for c in range(N_CORES)
    ]


def kernel(**inputs) -> np.ndarray:
    nc = build()
    in_maps = make_in_maps(inputs)
    res = run_bass_kernel_spmd(nc, in_maps, core_ids=list(range(N_CORES)))
    full = np.concatenate([res.results[c]["out"] for c in range(N_CORES)], axis=0)
    return full[None].astype(np.float32)


if __name__ == "__main__":
    rng = np.random.default_rng(0)
    s = 0.02
    inputs = {
        "x": rng.standard_normal((1, SEQ, DM), dtype=np.float32),
        "Wq": rng.standard_normal((DM, DM), dtype=np.float32) * s,
        "bq": rng.standard_normal((DM,), dtype=np.float32) * s,
        "Wk": rng.standard_normal((DM, KV), dtype=np.float32) * s,
        "bk": rng.standard_normal((KV,), dtype=np.float32) * s,
        "Wv": rng.standard_normal((DM, KV), dtype=np.float32) * s,
        "bv": rng.standard_normal((KV,), dtype=np.float32) * s,
        "Wo": rng.standard_normal((DM, DM), dtype=np.float32) * s,
        "bo": rng.standard_normal((DM,), dtype=np.float32) * s,
    }
    out = kernel(**inputs)
    print("out shape", out.shape, "finite", np.isfinite(out).all())
